# revision 1
# baseline (speedup 1.0000x reference)
"""C3DLoss kernel for Trainium2 — 8-core batch-parallel, raw-Bass, v2.

Per core = one batch frame b (pose partner tb = b^1):
    partial = sum over terms t in {same, cross}, shifts delta in [-2,2]^2,
              pixels p of
        exp(-50 * sum_c (ref_c(p) - q_c(p+delta))^2)
    with masks PRE-BAKED into the z channel on the host
    (query z += 30*(1-mq), ref z -= 30*(1-mg)  ->  masked pairs get
     d^2 >= ~900 so exp underflows to exactly 0).
    loss = -(sum of partials) / max(sum(depth_gt_mask), 1)

Device mapping (v2):
  - W split into G=64 blocks of WB=19 (+-2 halo -> WBH=23 cols); H in
    NSLAB=11 slabs of HS=32 rows.  6 feature channels paired as
    (x,y), (z,r), (g,b) -> per term a stacked [128, 3*plane] bf16 tile
    (3 channel-pair planes concatenated in the free dim); shifts are
    free-dim offsets in the haloed layout.
  - DVE: per (term,shift) slot ONE 3D-AP bf16 subtract (2x perf mode)
    into a packed stacked [128, 3*608] diff tile; the square (one
    stacked mul) is assigned per-slot to DVE, ACT (Square), or Pool by
    a load-balance table.
  - PE: 3 selector matmuls per chunk (K=128 = 2ch x 64blk, M=64 blocks)
    accumulate channel sums into PSUM; 2 chunks of 304 cols per slot;
    even/odd slot in PE-array cols / PSUM partitions 0-63 / 64-127.
  - ACT: exp(-50*x) in-place on PSUM spanning 4 banks (2 slot-pairs)
    per instruction with fused accum_out into per-EU accumulator cols.
  - Sems via embedded .then_inc on compute instrs (no drains).
"""

import os
import sys

for _p in ("/opt/trn_rl_repo", "/opt/pypackages"):
    if _p not in sys.path:
        sys.path.insert(0, _p)

from contextlib import ExitStack

import numpy as np
import ml_dtypes
from numpy.lib.stride_tricks import sliding_window_view

import concourse.bass as bass
import concourse.mybir as mybir
from concourse.ap import AP
from concourse.alu_op_type import AluOpType

# PARANOID inserts same-engine waits after producers so CoreSim's
# race detector (which only understands sem-based sync) can validate the
# program.  Engines are in-order, so production builds skip these waits.
PARANOID = os.environ.get("KV2_PARANOID") == "1"

F32 = mybir.dt.float32
BF16 = mybir.dt.bfloat16
BF_NP = ml_dtypes.bfloat16

R = 2
H, W = 352, 1216
G = 64
WB = W // G            # 19
WBH = WB + 2 * R       # 23
Hp = H + 2 * R         # 356
HS = 32
NSLAB = H // HS        # 11
NQP = Hp * WBH         # 8188 elems per partition per DRAM plane
QF = (HS + 2 * R) * WBH   # 828   query plane free size (per pair-plane)
RF = HS * WBH             # 736   ref plane free size
SQF = HS * WB             # 608   packed diff/sq plane free size
DFF = 3 * SQF             # 1824  stacked diff/sq tile free size
NCH = (HS // 2) * WB      # 304   psum chunk cols (16 rows)
BIG = 30.0
EXP_SCALE = -50.0

NSQ = 16    # sq ring (slots)
NDF = 8     # diff ring (slots)
LA = 4      # ACT square lookahead (slots)

NSLOT = NSLAB * 50        # 550
NPAIR = NSLOT // 2        # 275
NUNIT = 2 * NPAIR         # 550 psum chunk-units
EUN = 4                   # psum chunk-units per exp instruction
NEU = (NUNIT + EUN - 1) // EUN   # exp units
NACC = 288                # acc cols (padded)

SHIFTS = [(t, dy, dx) for t in (0, 1)
          for dy in range(-R, R + 1) for dx in range(-R, R + 1)]


# Square assignment: mostly whole-slot (3-plane) ops to amortize fixed
# per-op costs; a few split slots hit the balance-solve plane ratio
# D:A:P = 13:36:23 per 72 (sim-swept; engines land ~661/658/657 us).
_A3 = [("A", 0, 3)]
_P3 = [("P", 0, 3)]
_D3 = [("D", 0, 3)]
_PD = [("P", 0, 2), ("D", 2, 1)]
_AD = [("A", 0, 2), ("D", 2, 1)]
_DP = [("D", 0, 2), ("P", 2, 1)]
_DA = [("D", 0, 1), ("A", 1, 2)]
_CYC = ([_A3, _P3, _D3, _A3, _PD, _A3, _P3, _A3, _AD, _P3, _A3, _DA]
        + [_A3, _P3, _D3, _A3, _PD, _A3, _P3, _A3, _AD, _P3, _A3, _DP])
PLAN = []
SQLIST = {}
ORD2 = {}


def set_plan(cyc):
    """(Re)build the square-producer schedule from a cycle of slot plans."""
    global PLAN, SQLIST, ORD2
    PLAN = [cyc[k % len(cyc)] for k in range(NSLOT)]
    SQLIST = {"D": [], "A": [], "P": []}
    ORD2 = {}
    for k in range(NSLOT):
        for (e, lo, np_) in PLAN[k]:
            ORD2[(k, e)] = len(SQLIST[e])
            SQLIST[e].append((k, lo, np_))
    _CACHED.clear()


_CACHED = {}
set_plan(_CYC)


def _apv(t_ap, p0, pcnt, free_dims, free_off=0):
    pstride = t_ap.ap[0][0]
    base = t_ap.offset + p0 * pstride + free_off
    return AP(t_ap.tensor, base, [[pstride, pcnt]] + [list(d) for d in free_dims])


def _dram_ap(handle, offset, dims):
    a = handle[:]
    return AP(a.tensor, a.offset + offset, [list(d) for d in dims])


def make_sel():
    s = np.zeros((128, G), dtype=BF_NP)
    for c in range(2):
        for g in range(G):
            s[c * G + g, g] = 1
    return s


def emit(nc: bass.Bass):
    Act = mybir.ActivationFunctionType
    dp = nc.declare_dram_parameter
    q_d = dp("q_d", [2, 128, 3 * NQP], BF16, isOutput=False)
    r_d = dp("r_d", [2, 128, 3 * NQP], BF16, isOutput=False)
    sel_d = dp("sel_d", [128, G], BF16, isOutput=False)
    res_d = dp("res_d", [128, 1], F32, isOutput=True)

    with ExitStack() as ex:
        E = ex.enter_context
        qt = [[E(nc.sbuf_tensor(f"qt{t}{p}", [128, 3 * QF], BF16))
               for p in range(2)] for t in range(2)]
        rt = [[E(nc.sbuf_tensor(f"rt{t}{p}", [128, 3 * RF], BF16))
               for p in range(2)] for t in range(2)]
        df = [E(nc.sbuf_tensor(f"df{i}", [128, DFF], BF16)) for i in range(NDF)]
        sq = [E(nc.sbuf_tensor(f"sq{i}", [128, DFF], BF16)) for i in range(NSQ)]
        sel_s = E(nc.sbuf_tensor("sel", [128, G], BF16))
        acc_s = E(nc.sbuf_tensor("acc", [128, NACC], F32))
        res_s = E(nc.sbuf_tensor("res", [128, 1], F32))
        pst = E(nc.psum_tensor("pst", [128, 4096], F32))

        sLC = E(nc.semaphore("sLC"))
        # per-phase, per-term load sems: DMA completion order across queues
        # is not guaranteed, so each wait threshold must count only the DMAs
        # it actually needs
        sLt = [[E(nc.semaphore(f"sL{t}{p}")) for p in range(2)]
               for t in range(2)]
        sG = E(nc.semaphore("sG"))
        sV = E(nc.semaphore("sV"))    # DVE subs, 1/slot (+ final reduce)
        sQD = E(nc.semaphore("sQD"))  # squares done, by producer
        sQA = E(nc.semaphore("sQA"))
        sQP = E(nc.semaphore("sQP"))
        sP = E(nc.semaphore("sP"))    # PE: 2 per slot (per chunk)
        sA = E(nc.semaphore("sA"))    # ACT exp EUs
        blk = E(nc.Block())

        semQ = {"D": sQD, "A": sQA, "P": sQP}

        def sq_ring_wait(eng, k):
            # sq[k%NSQ] previously held slot k-NSQ; wait for its PE consumer
            if k >= NSQ:
                eng.wait_ge(sP, 2 * (k - NSQ) + 2)

        def sq_done_wait(eng, k):
            # wait for every producer of slot k's squares
            for (e, _lo, _np) in PLAN[k]:
                eng.wait_ge(semQ[e], ORD2[(k, e)] + 1)

        @blk.gpsimd
        def _(gp):
            gp.memset(acc_s.ap(), 0.0)
            gp.drain()
            gp.sem_inc(sG, 1)
            for (k, lo, np_) in SQLIST["P"]:
                gp.wait_ge(sV, k + 1)
                sq_ring_wait(gp, k)
                gp.tensor_mul(
                    _apv(sq[k % NSQ].ap(), 0, 128, [[1, np_ * SQF]], lo * SQF),
                    _apv(df[k % NDF].ap(), 0, 128, [[1, np_ * SQF]], lo * SQF),
                    _apv(df[k % NDF].ap(), 0, 128, [[1, np_ * SQF]], lo * SQF),
                ).then_inc(sQP, 1)

        @blk.sync
        def _(sp):
            sp.dma_start(sel_s[:], sel_d[:]).then_inc(sLC, 16)
            for s in range(NSLAB):
                ph = s % 2
                if s >= 2:
                    sp.wait_ge(sV, 50 * (s - 1))
                r0 = s * HS
                for t in range(2):
                    sp.dma_start(
                        qt[t][ph].ap(),
                        _dram_ap(q_d, t * 128 * 3 * NQP + r0 * WBH,
                                 [[3 * NQP, 128], [NQP, 3], [1, QF]])
                    ).then_inc(sLt[t][ph], 16)
                    sp.dma_start(
                        rt[t][ph].ap(),
                        _dram_ap(r_d, t * 128 * 3 * NQP + (r0 + 2) * WBH,
                                 [[3 * NQP, 128], [NQP, 3], [1, RF]])
                    ).then_inc(sLt[t][ph], 16)
            sp.wait_ge(sV, NSLOT + 1)
            sp.dma_start(res_d[:], res_s.ap()).then_inc(sLC, 16)

        @blk.vector
        def _(ve):
            ve.wait_ge(sLC, 16)
            for k in range(NSLOT):
                s = k // 50
                ph = s % 2
                if k % 50 == 0:
                    ve.wait_ge(sLt[0][ph], 32 * (s // 2 + 1))
                elif k % 50 == 25:
                    ve.wait_ge(sLt[1][ph], 32 * (s // 2 + 1))
                term, dy, dx = SHIFTS[k % 50]
                qoff = (2 + dy) * WBH + (2 + dx)
                if k >= NDF:
                    sq_done_wait(ve, k - NDF)
                nc.vector.tensor_tensor(
                    _apv(df[k % NDF].ap(), 0, 128, [[SQF, 3], [WB, HS], [1, WB]]),
                    _apv(rt[term][ph].ap(), 0, 128, [[RF, 3], [WBH, HS], [1, WB]], 2),
                    _apv(qt[term][ph].ap(), 0, 128, [[QF, 3], [WBH, HS], [1, WB]], qoff),
                    AluOpType.subtract).then_inc(sV, 1)
                for (e, lo, np_) in PLAN[k]:
                    if e != "D":
                        continue
                    sq_ring_wait(ve, k)
                    if PARANOID:
                        ve.wait_ge(sV, k + 1)
                    nc.vector.tensor_mul(
                        _apv(sq[k % NSQ].ap(), 0, 128, [[1, np_ * SQF]], lo * SQF),
                        _apv(df[k % NDF].ap(), 0, 128, [[1, np_ * SQF]], lo * SQF),
                        _apv(df[k % NDF].ap(), 0, 128, [[1, np_ * SQF]], lo * SQF),
                    ).then_inc(sQD, 1)
            ve.wait_ge(sA, NEU)
            nc.vector.tensor_reduce(
                res_s.ap(), acc_s.ap(), axis=mybir.AxisListType.X,
                op=AluOpType.add).then_inc(sV, 1)

        @blk.tensor
        def _(pe):
            pe.wait_ge(sLC, 16)
            for k in range(NSLOT):
                par = k % 2
                pr = k // 2
                if par == 0 and 2 * pr >= 8:
                    # banks (2p)%8,(2p+1)%8 freed when EU (2p-8)//EUN expd
                    pe.wait_ge(sA, (2 * pr - 8) // EUN + 1)
                sq_done_wait(pe, k)
                for c in range(2):
                    u = 2 * pr + c
                    col = 512 * (u % 8)
                    for t in range(3):
                        mm = nc.tensor.matmul(
                            pst[G * par:G * par + G, col:col + NCH], sel_s[:],
                            _apv(sq[k % NSQ].ap(), 0, 128, [[1, NCH]],
                                 t * SQF + NCH * c),
                            start=(t == 0), stop=(t == 2),
                            skip_group_check=True,
                            tile_position=(0, G * par))
                        if t == 2:
                            mm.then_inc(sP, 1)

        @blk.scalar
        def _(ac):
            ac.wait_ge(sG, 1)
            aptr = 0

            act_list = SQLIST["A"]

            def flush(upto_slot, aptr):
                while aptr < len(act_list) and act_list[aptr][0] < upto_slot:
                    k, lo, np_ = act_list[aptr]
                    ac.wait_ge(sV, k + 1)
                    sq_ring_wait(ac, k)
                    nc.scalar.activation(
                        _apv(sq[k % NSQ].ap(), 0, 128, [[1, np_ * SQF]], lo * SQF),
                        _apv(df[k % NDF].ap(), 0, 128, [[1, np_ * SQF]], lo * SQF),
                        mybir.ActivationFunctionType.Square).then_inc(sQA, 1)
                    aptr += 1
                return aptr

            for e in range(NEU):
                u0 = EUN * e
                nu = min(EUN, NUNIT - u0)
                last_slot = 2 * ((u0 + nu - 1) // 2) + 1  # last slot of EU
                aptr = flush(min(last_slot + 1 + LA, NSLOT), aptr)
                ac.wait_ge(sP, 2 * (2 * ((u0 + nu - 1) // 2) + 2))
                col = 512 * (u0 % 8)
                ap_io = AP(pst[:].tensor, pst[:].offset + col,
                           [[pst[:].ap[0][0], 128], [512, nu], [1, NCH]])
                nc.scalar.activation(
                    ap_io, ap_io,
                    mybir.ActivationFunctionType.Exp,
                    scale=EXP_SCALE,
                    accum_out=acc_s[:, e:e + 1]).then_inc(sA, 1)
            flush(NSLOT, aptr)
    return nc


# ---------------- host side ----------------

def _pad(x, fill=0.0):
    p = np.full((Hp, W + 2 * R), fill, np.float32)
    p[R:R + H, R:R + W] = x
    return p


def _block_tiles(planes):
    """6 padded [Hp, W+4] planes (P0c0,P0c1,P1c0,P1c1,P2c0,P2c1)
    -> [128, 3*NQP] bf16: partition = c*64+g, free = plane*NQP + row*WBH+col."""
    P = np.stack(planes)                                   # [6, Hp, Wp]
    sw = sliding_window_view(P, WBH, axis=2)[:, :, ::WB]   # [6, Hp, G, WBH]
    blocked = np.ascontiguousarray(sw.transpose(0, 2, 1, 3))  # [6, G, Hp, WBH]
    b = blocked.reshape(3, 2, G, NQP).transpose(1, 2, 0, 3)   # [2, G, 3, NQP]
    return b.astype(BF_NP).reshape(128, 3 * NQP)


def host_precompute(rgb, depth, depth_gt, depth_mask, depth_gt_mask,
                    xy1_grid, Ts, b):
    tb = b ^ 1
    f32 = np.float32
    xy1 = np.asarray(xy1_grid[b], f32)
    dep = np.asarray(depth[b, 0], f32)
    dgt_b = np.asarray(depth_gt[b, 0], f32)
    dgt_t = np.asarray(depth_gt[tb, 0], f32)
    mp = np.asarray(depth_mask[b, 0], f32)
    mg_b = np.asarray(depth_gt_mask[b, 0], f32)
    mg_t = np.asarray(depth_gt_mask[tb, 0], f32)
    rgb_b = np.asarray(rgb[b], f32)
    rgb_t = np.asarray(rgb[tb], f32)

    xyz_p = xy1 * dep
    xyz_gb = xy1 * dgt_b
    xyz_gt = xy1 * dgt_t
    T21 = (np.linalg.inv(np.asarray(Ts[tb], np.float64)) @
           np.asarray(Ts[b], np.float64)).astype(f32)
    Rm, tv = T21[:3, :3], T21[:3, 3]
    txyz = np.einsum('ij,jhw->ihw', Rm, xyz_p).astype(f32) \
        + tv[:, None, None].astype(f32)
    posq = (txyz[2] > 0).astype(f32) * mp

    qz0 = xyz_p[2] + BIG * (1.0 - mp)
    qz1 = txyz[2] + BIG * (1.0 - posq)
    rz0 = xyz_gb[2] - BIG * (1.0 - mg_b)
    rz1 = xyz_gt[2] - BIG * (1.0 - mg_t)

    # per term: planes (x,y), (z,r), (g,b); query z has masks baked +BIG,
    # ref z has masks baked -BIG
    p_rb = [_pad(rgb_b[0]), _pad(rgb_b[1]), _pad(rgb_b[2])]
    p_rt = [_pad(rgb_t[0]), _pad(rgb_t[1]), _pad(rgb_t[2])]
    q = np.stack([
        _block_tiles([_pad(xyz_p[0]), _pad(xyz_p[1]),
                      _pad(qz0, BIG), p_rb[0], p_rb[1], p_rb[2]]),
        _block_tiles([_pad(txyz[0]), _pad(txyz[1]),
                      _pad(qz1, BIG), p_rb[0], p_rb[1], p_rb[2]]),
    ])
    r = np.stack([
        _block_tiles([_pad(xyz_gb[0]), _pad(xyz_gb[1]),
                      _pad(rz0), p_rb[0], p_rb[1], p_rb[2]]),
        _block_tiles([_pad(xyz_gt[0]), _pad(xyz_gt[1]),
                      _pad(rz1), p_rt[0], p_rt[1], p_rt[2]]),
    ])
    return {"q_d": q, "r_d": r, "sel_d": make_sel()}


def make_in_maps(rgb, depth, depth_gt, depth_mask, depth_gt_mask, xy1_grid, Ts,
                 n_cores=8):
    return [host_precompute(rgb, depth, depth_gt, depth_mask, depth_gt_mask,
                            xy1_grid, Ts, b) for b in range(n_cores)]


def _get_nc():
    if "nc" not in _CACHED:
        nc = bass.Bass()
        emit(nc)
        _CACHED["nc"] = nc
    return _CACHED["nc"]


def kernel(rgb, depth, depth_gt, depth_mask, depth_gt_mask, xy1_grid, Ts,
           **run_kwargs):
    from concourse.bass_utils import run_bass_kernel_spmd
    nc = _get_nc()
    maps = make_in_maps(rgb, depth, depth_gt, depth_mask, depth_gt_mask,
                        xy1_grid, Ts)
    res = run_bass_kernel_spmd(nc, maps, list(range(8)), **run_kwargs)
    total = np.float64(0.0)
    for r in res.results:
        total += np.float64(r["res_d"][:, 0].sum())
    n_gt = max(np.asarray(depth_gt_mask, np.float64).sum(), 1.0)
    loss = -total / n_gt
    kernel.last_results = res
    return np.float32(loss)



# revision 2
# speedup vs baseline: 1.0068x; 1.0068x over previous
"""C3DLoss kernel for Trainium2 — 8-core batch-parallel, raw-Bass, v9.

v9 = baseline v2 pipeline (slot-granular squares/PE/exp, swept plan)
with ONE structural change: the DVE subtract covers BOTH terms of a
shift in a single 6-plane instruction (halves sub instruction count and
fixed access overhead; DVE -16us).  Slot order is re-paired so the two
terms of one shift are adjacent slots (even=term0, odd=term1), which
also merges the per-slab DMA into one q + one r load.

Per core = one batch frame b (pose partner tb = b^1):
    partial = sum over terms t in {same, cross}, shifts delta in
              [-2,2]^2, pixels p of
        exp(-50 * sum_c (ref_c(p) - q_c(p+delta))^2)
    with masks PRE-BAKED into the z channel on the host.
    loss = -(sum of partials) / max(sum(depth_gt_mask), 1)
"""

import os
import sys

for _p in ("/opt/trn_rl_repo", "/opt/pypackages"):
    if _p not in sys.path:
        sys.path.insert(0, _p)

from contextlib import ExitStack

import numpy as np
import ml_dtypes
from numpy.lib.stride_tricks import sliding_window_view

import concourse.bass as bass
import concourse.mybir as mybir
from concourse.ap import AP
from concourse.alu_op_type import AluOpType

PARANOID = os.environ.get("KV9_PARANOID") == "1"

F32 = mybir.dt.float32
BF16 = mybir.dt.bfloat16
BF_NP = ml_dtypes.bfloat16

R = 2
H, W = 352, 1216
G = 64
WB = W // G            # 19
WBH = WB + 2 * R       # 23
Hp = H + 2 * R         # 356
HS = 32
NSLAB = H // HS        # 11
NQP = Hp * WBH         # 8188
QF = (HS + 2 * R) * WBH   # 828
RF = HS * WBH             # 736
SQF = HS * WB             # 608
SLF = 3 * SQF             # 1824  per-slot (3-plane) free size
DFF = 6 * SQF             # 3648  per-job (6-plane) df free size
NCH = (HS // 2) * WB      # 304
BIG = 30.0
EXP_SCALE = -50.0

NSQ = 16    # sq ring (slots)
NDF = 4     # df ring (jobs)
LA = 4      # ACT square lookahead (slots)

NSLOT = NSLAB * 50        # 550 (slot k: term=k%2, shift=(k%50)//2)
NJOB = NSLOT // 2         # 275
NUNIT = NSLOT             # psum chunk-units
EUN = 4
NEU = (NUNIT + EUN - 1) // EUN
NACC = 288

SHIFTS = [(dy, dx) for dy in range(-R, R + 1) for dx in range(-R, R + 1)]

# Square assignment per slot (3 planes), same machinery as v2 but with
# the plan re-swept for the lower DVE load: D:A:P = 16:35:21 per 72.
_A3 = [("A", 0, 3)]
_P3 = [("P", 0, 3)]
_D3 = [("D", 0, 3)]
_PD = [("P", 0, 2), ("D", 2, 1)]
_AD = [("A", 0, 2), ("D", 2, 1)]
_DP = [("D", 0, 2), ("P", 2, 1)]
_DA = [("D", 0, 1), ("A", 1, 2)]
# 48-slot cycle: first 12 rebalanced (_AD/_PD heavier on DVE) for the
# lower DVE load after the term-merged subs; D:A:P = 28:71:45 per 144.
_CYC = ([_A3, _P3, _D3, _A3, _PD, _A3, _P3, _A3, _AD, _P3, _A3, _DA]
        + [_A3, _P3, _D3, _A3, _PD, _A3, _P3, _A3, _AD, _P3, _A3, _DP]
        + [_AD, _PD, _D3, _A3, _PD, _A3, _P3, _A3, _AD, _P3, _A3, _DA]
        + [_A3, _P3, _D3, _A3, _PD, _A3, _P3, _A3, _AD, _P3, _A3, _DP])
PLAN = []
SQLIST = {}
ORD2 = {}


def set_plan(cyc):
    global PLAN, SQLIST, ORD2
    PLAN = [cyc[k % len(cyc)] for k in range(NSLOT)]
    SQLIST = {"D": [], "A": [], "P": []}
    ORD2 = {}
    for k in range(NSLOT):
        for (e, lo, np_) in PLAN[k]:
            ORD2[(k, e)] = len(SQLIST[e])
            SQLIST[e].append((k, lo, np_))
    _CACHED.clear()


_CACHED = {}
set_plan(_CYC)


def _apv(t_ap, p0, pcnt, free_dims, free_off=0):
    pstride = t_ap.ap[0][0]
    base = t_ap.offset + p0 * pstride + free_off
    return AP(t_ap.tensor, base, [[pstride, pcnt]] + [list(d) for d in free_dims])


def _dram_ap(handle, offset, dims):
    a = handle[:]
    return AP(a.tensor, a.offset + offset, [list(d) for d in dims])


def make_sel():
    s = np.zeros((128, G), dtype=BF_NP)
    for c in range(2):
        for g in range(G):
            s[c * G + g, g] = 1
    return s


def emit(nc: bass.Bass):
    dp = nc.declare_dram_parameter
    q_d = dp("q_d", [128, 6 * NQP], BF16, isOutput=False)
    r_d = dp("r_d", [128, 6 * NQP], BF16, isOutput=False)
    sel_d = dp("sel_d", [128, G], BF16, isOutput=False)
    res_d = dp("res_d", [128, 1], F32, isOutput=True)

    with ExitStack() as ex:
        E = ex.enter_context
        qt = [E(nc.sbuf_tensor(f"qt{p}", [128, 6 * QF], BF16))
              for p in range(2)]
        rt = [E(nc.sbuf_tensor(f"rt{p}", [128, 6 * RF], BF16))
              for p in range(2)]
        df = E(nc.sbuf_tensor("df", [128, NDF * DFF], BF16))
        sq = [E(nc.sbuf_tensor(f"sq{i}", [128, SLF], BF16)) for i in range(NSQ)]
        sel_s = E(nc.sbuf_tensor("sel", [128, G], BF16))
        acc_s = E(nc.sbuf_tensor("acc", [128, NACC], F32))
        res_s = E(nc.sbuf_tensor("res", [128, 1], F32))
        pst = E(nc.psum_tensor("pst", [128, 4096], F32))

        sLC = E(nc.semaphore("sLC"))
        sLt = [E(nc.semaphore(f"sL{p}")) for p in range(2)]
        sV = E(nc.semaphore("sV"))    # DVE subs, 1/JOB (+ final reduce)
        sQD = E(nc.semaphore("sQD"))
        sQA = E(nc.semaphore("sQA"))
        sQP = E(nc.semaphore("sQP"))
        sP = E(nc.semaphore("sP"))    # PE: 2 per slot (per chunk)
        sA = E(nc.semaphore("sA"))    # ACT exp EUs
        blk = E(nc.Block())

        semQ = {"D": sQD, "A": sQA, "P": sQP}

        def df_ap(k, lo, np_):
            # slot k's 3-plane half inside job tile (k//2)%NDF
            off = ((k // 2) % NDF) * DFF + (k % 2) * SLF + lo * SQF
            return _apv(df.ap(), 0, 128, [[1, np_ * SQF]], off)

        def sub_wait(eng, k):
            eng.wait_ge(sV, k // 2 + 1)

        def sq_ring_wait(eng, k):
            if k >= NSQ:
                eng.wait_ge(sP, 2 * (k - NSQ) + 2)

        def sq_done_wait(eng, k):
            for (e, _lo, _np) in PLAN[k]:
                eng.wait_ge(semQ[e], ORD2[(k, e)] + 1)

        @blk.gpsimd
        def _(gp):
            gp.memset(acc_s.ap(), 0.0)
            gp.drain()
            gp.sem_inc(sLC, 1)
            for (k, lo, np_) in SQLIST["P"]:
                gp.wait_ge(sV, k // 2 + 1)
                sq_ring_wait(gp, k)
                gp.tensor_mul(
                    _apv(sq[k % NSQ].ap(), 0, 128, [[1, np_ * SQF]], lo * SQF),
                    df_ap(k, lo, np_),
                    df_ap(k, lo, np_),
                ).then_inc(sQP, 1)

        @blk.sync
        def _(sp):
            sp.dma_start(sel_s[:], sel_d[:]).then_inc(sLC, 16)
            for s in range(NSLAB):
                ph = s % 2
                if s >= 2:
                    sp.wait_ge(sV, 25 * (s - 1))
                r0 = s * HS
                sp.dma_start(
                    qt[ph].ap(),
                    _dram_ap(q_d, r0 * WBH,
                             [[6 * NQP, 128], [NQP, 6], [1, QF]])
                ).then_inc(sLt[ph], 16)
                sp.dma_start(
                    rt[ph].ap(),
                    _dram_ap(r_d, (r0 + 2) * WBH,
                             [[6 * NQP, 128], [NQP, 6], [1, RF]])
                ).then_inc(sLt[ph], 16)
            sp.wait_ge(sV, NJOB + 1)
            sp.dma_start(res_d[:], res_s.ap()).then_inc(sLC, 16)

        @blk.vector
        def _(ve):
            ve.wait_ge(sLC, 16)
            for k in range(NSLOT):
                s = k // 50
                ph = s % 2
                if k % 50 == 0:
                    ve.wait_ge(sLt[ph], 32 * (s // 2 + 1))
                if k % 2 == 0:
                    # one 6-plane sub covers both terms of this shift
                    j = k // 2
                    dy, dx = SHIFTS[(k % 50) // 2]
                    qoff = (2 + dy) * WBH + (2 + dx)
                    if j >= NDF:
                        sq_done_wait(ve, 2 * (j - NDF))
                        sq_done_wait(ve, 2 * (j - NDF) + 1)
                    nc.vector.tensor_tensor(
                        _apv(df.ap(), 0, 128,
                             [[SQF, 6], [WB, HS], [1, WB]],
                             (j % NDF) * DFF),
                        _apv(rt[ph].ap(), 0, 128,
                             [[RF, 6], [WBH, HS], [1, WB]], 2),
                        _apv(qt[ph].ap(), 0, 128,
                             [[QF, 6], [WBH, HS], [1, WB]], qoff),
                        AluOpType.subtract).then_inc(sV, 1)
                for (e, lo, np_) in PLAN[k]:
                    if e != "D":
                        continue
                    sq_ring_wait(ve, k)
                    if PARANOID:
                        ve.wait_ge(sV, k // 2 + 1)
                    nc.vector.tensor_mul(
                        _apv(sq[k % NSQ].ap(), 0, 128, [[1, np_ * SQF]],
                             lo * SQF),
                        df_ap(k, lo, np_),
                        df_ap(k, lo, np_),
                    ).then_inc(sQD, 1)
            ve.wait_ge(sA, NEU)
            nc.vector.tensor_reduce(
                res_s.ap(), acc_s.ap(), axis=mybir.AxisListType.X,
                op=AluOpType.add).then_inc(sV, 1)

        @blk.tensor
        def _(pe):
            pe.wait_ge(sLC, 16)
            for k in range(NSLOT):
                par = k % 2
                pr = k // 2
                if par == 0 and 2 * pr >= 8:
                    pe.wait_ge(sA, (2 * pr - 8) // EUN + 1)
                sq_done_wait(pe, k)
                for c in range(2):
                    u = 2 * pr + c
                    col = 512 * (u % 8)
                    for t in range(3):
                        mm = nc.tensor.matmul(
                            pst[G * par:G * par + G, col:col + NCH], sel_s[:],
                            _apv(sq[k % NSQ].ap(), 0, 128, [[1, NCH]],
                                 t * SQF + NCH * c),
                            start=(t == 0), stop=(t == 2),
                            skip_group_check=True,
                            tile_position=(0, G * par))
                        if t == 2:
                            mm.then_inc(sP, 1)

        @blk.scalar
        def _(ac):
            ac.wait_ge(sLC, 17)
            aptr = 0

            act_list = SQLIST["A"]

            def flush(upto_slot, aptr):
                while aptr < len(act_list) and act_list[aptr][0] < upto_slot:
                    k, lo, np_ = act_list[aptr]
                    ac.wait_ge(sV, k // 2 + 1)
                    sq_ring_wait(ac, k)
                    nc.scalar.activation(
                        _apv(sq[k % NSQ].ap(), 0, 128, [[1, np_ * SQF]],
                             lo * SQF),
                        df_ap(k, lo, np_),
                        mybir.ActivationFunctionType.Square).then_inc(sQA, 1)
                    aptr += 1
                return aptr

            for e in range(NEU):
                u0 = EUN * e
                nu = min(EUN, NUNIT - u0)
                last_slot = 2 * ((u0 + nu - 1) // 2) + 1
                aptr = flush(min(last_slot + 1 + LA, NSLOT), aptr)
                ac.wait_ge(sP, 2 * (2 * ((u0 + nu - 1) // 2) + 2))
                col = 512 * (u0 % 8)
                ap_io = AP(pst[:].tensor, pst[:].offset + col,
                           [[pst[:].ap[0][0], 128], [512, nu], [1, NCH]])
                nc.scalar.activation(
                    ap_io, ap_io,
                    mybir.ActivationFunctionType.Exp,
                    scale=EXP_SCALE,
                    accum_out=acc_s[:, e:e + 1]).then_inc(sA, 1)
            flush(NSLOT, aptr)
    return nc


# ---------------- host side ----------------

def _pad(x, fill=0.0):
    p = np.full((Hp, W + 2 * R), fill, np.float32)
    p[R:R + H, R:R + W] = x
    return p


def _block_tiles(planes):
    flat = []
    for c0, c1 in planes:
        flat.append(c0)
        flat.append(c1)
    P = np.stack(flat)                                     # [12, Hp, Wp]
    sw = sliding_window_view(P, WBH, axis=2)[:, :, ::WB]   # [12, Hp, G, WBH]
    blocked = np.ascontiguousarray(sw.transpose(0, 2, 1, 3))
    b = blocked.reshape(6, 2, G, NQP).transpose(1, 2, 0, 3)
    return b.astype(BF_NP).reshape(128, 6 * NQP)


def host_precompute(rgb, depth, depth_gt, depth_mask, depth_gt_mask,
                    xy1_grid, Ts, b):
    tb = b ^ 1
    f32 = np.float32
    xy1 = np.asarray(xy1_grid[b], f32)
    dep = np.asarray(depth[b, 0], f32)
    dgt_b = np.asarray(depth_gt[b, 0], f32)
    dgt_t = np.asarray(depth_gt[tb, 0], f32)
    mp = np.asarray(depth_mask[b, 0], f32)
    mg_b = np.asarray(depth_gt_mask[b, 0], f32)
    mg_t = np.asarray(depth_gt_mask[tb, 0], f32)
    rgb_b = np.asarray(rgb[b], f32)
    rgb_t = np.asarray(rgb[tb], f32)

    xyz_p = xy1 * dep
    xyz_gb = xy1 * dgt_b
    xyz_gt = xy1 * dgt_t
    T21 = (np.linalg.inv(np.asarray(Ts[tb], np.float64)) @
           np.asarray(Ts[b], np.float64)).astype(f32)
    Rm, tv = T21[:3, :3], T21[:3, 3]
    txyz = np.einsum('ij,jhw->ihw', Rm, xyz_p).astype(f32) \
        + tv[:, None, None].astype(f32)
    posq = (txyz[2] > 0).astype(f32) * mp

    qz0 = xyz_p[2] + BIG * (1.0 - mp)
    qz1 = txyz[2] + BIG * (1.0 - posq)
    rz0 = xyz_gb[2] - BIG * (1.0 - mg_b)
    rz1 = xyz_gt[2] - BIG * (1.0 - mg_t)

    p_rb = [_pad(rgb_b[0]), _pad(rgb_b[1]), _pad(rgb_b[2])]
    p_rt = [_pad(rgb_t[0]), _pad(rgb_t[1]), _pad(rgb_t[2])]
    q = _block_tiles([
        (_pad(xyz_p[0]), _pad(xyz_p[1])),
        (_pad(qz0, BIG), p_rb[0]),
        (p_rb[1], p_rb[2]),
        (_pad(txyz[0]), _pad(txyz[1])),
        (_pad(qz1, BIG), p_rb[0]),
        (p_rb[1], p_rb[2]),
    ])
    r = _block_tiles([
        (_pad(xyz_gb[0]), _pad(xyz_gb[1])),
        (_pad(rz0), p_rb[0]),
        (p_rb[1], p_rb[2]),
        (_pad(xyz_gt[0]), _pad(xyz_gt[1])),
        (_pad(rz1), p_rt[0]),
        (p_rt[1], p_rt[2]),
    ])
    return {"q_d": q, "r_d": r, "sel_d": make_sel()}


def make_in_maps(rgb, depth, depth_gt, depth_mask, depth_gt_mask, xy1_grid, Ts,
                 n_cores=8):
    return [host_precompute(rgb, depth, depth_gt, depth_mask, depth_gt_mask,
                            xy1_grid, Ts, b) for b in range(n_cores)]


def _get_nc():
    if "nc" not in _CACHED:
        nc = bass.Bass()
        emit(nc)
        _CACHED["nc"] = nc
    return _CACHED["nc"]


def kernel(rgb, depth, depth_gt, depth_mask, depth_gt_mask, xy1_grid, Ts,
           **run_kwargs):
    from concourse.bass_utils import run_bass_kernel_spmd
    nc = _get_nc()
    maps = make_in_maps(rgb, depth, depth_gt, depth_mask, depth_gt_mask,
                        xy1_grid, Ts)
    res = run_bass_kernel_spmd(nc, maps, list(range(8)), **run_kwargs)
    total = np.float64(0.0)
    for r in res.results:
        total += np.float64(r["res_d"][:, 0].sum())
    n_gt = max(np.asarray(depth_gt_mask, np.float64).sum(), 1.0)
    loss = -total / n_gt
    kernel.last_results = res
    return np.float32(loss)
